# revision 19
# baseline (speedup 1.0000x reference)
"""Trainium2 Bass kernel for nn_Attention_14370960572643 (gnn_message_passing).

Math (per batch b):
  local_pair[b,i,j,:] = local[b,i,:] + local[b,j,:]
  att  = relu(concat(local_pair, binary) @ W1 + b1)        [B,N,N,H]
  score = sigmoid(att @ W2 + b2)                            [B,N,N,1]
  G[b,i,:] = sum_j local[b,j,:] * score[b,i,j]              [B,N,H]
  outputs (E sparse pairs): lp[e] = local[bb,ii]+local[bb,jj]
                            gp[e] = G[bb,ii]+G[bb,jj]

Key tricks:
  * local_pair @ W1a = P[b,i,:] + P[b,j,:] with P = local @ W1[:H] — the
    [B*N*N, 311] einsum collapses into ONE K=116 matmul per h-tile:
    lhsT rows 0..99 hold P (j term), rows 100..104 hold P+b1 for the
    chunk's 5 i values, rows 105..115 hold W1b; the rhs pairs those with
    a constant 0/1 indicator (rows 0..104) and the chunk's binary
    features (rows 105..115). PSUM gets Q + P_j + P_i + b1 in one pass;
    the epilogue is a single relu (split across DVE and ACT).
  * sparse gathers lp/gp are one-hot matmuls (the ii+jj add comes free);
    sparse entries are grouped by batch so each 128-row tile needs one
    matmul, and the batch-0 half of gp runs overlapped with batch-1
    compute.
  * scoreT ([j,i] layout for the G matmul) is produced by a strided
    sigmoid write plus one partition-scatter DMA per batch.

Sharding: data-parallel over B, 2 batches per core, 8 cores.
sparse_idx entries are routed to the core owning their batch.
"""

import numpy as np

B, N, H, BIN = 16, 100, 300, 11
HB = H + BIN  # 311
KC = 116                   # combined contraction: 100 P + 5 Pb + 11 W1b
NCORES = 8
BPC = B // NCORES          # batches per core
CAP_B = 1536               # padded sparse entries per (core, batch)
NT_B = CAP_B // 128        # 12 gather tiles per batch
CAP = CAP_B * BPC          # 3072 per core
NT = NT_B * BPC
CH_I = 5                   # i values per chunk
CH = CH_I * N              # 500 rows per chunk
NCH = N // CH_I            # 20 chunks per batch
H_T = [(0, 128), (128, 128), (256, 44)]   # h tiles (also used for k over H)

_CACHE = {}


def _build_nc():
    import concourse.bass as bass
    import concourse.mybir as mybir
    import concourse.tile as tile
    from concourse import bacc

    dt = mybir.dt
    f32 = dt.float32
    bf16 = dt.bfloat16

    nc = bacc.Bacc("TRN2", target_bir_lowering=False, debug=False,
                   num_devices=NCORES)

    # ---- dram parameters (per-core shards) ----
    localT = nc.dram_tensor("localT", [BPC, H, N], f32, kind="ExternalInput").ap()
    lnat16 = nc.dram_tensor("lnat16", [BPC * N, H], bf16, kind="ExternalInput").ap()
    binT = nc.dram_tensor("binT", [BPC, BIN, N * N], bf16, kind="ExternalInput").ap()
    W1d = nc.dram_tensor("W1", [HB, H], f32, kind="ExternalInput").ap()
    W1b16d = nc.dram_tensor("W1b16", [BIN, H], bf16, kind="ExternalInput").ap()
    W216d = nc.dram_tensor("W216", [H, 1], bf16, kind="ExternalInput").ap()
    b1d = nc.dram_tensor("b1", [1, H], f32, kind="ExternalInput").ap()
    b2d = nc.dram_tensor("b2", [1, 1], f32, kind="ExternalInput").ap()
    indJ5d = nc.dram_tensor("indJ5", [N + CH_I, CH], bf16,
                            kind="ExternalInput").ap()
    ohd = nc.dram_tensor("oh", [N, CAP], bf16, kind="ExternalInput").ap()
    lpd = nc.dram_tensor("lp", [CAP, H], bf16, kind="ExternalOutput").ap()
    gpd = nc.dram_tensor("gp", [CAP, H], bf16, kind="ExternalOutput").ap()

    Relu = mybir.ActivationFunctionType.Relu
    Sigmoid = mybir.ActivationFunctionType.Sigmoid

    with tile.TileContext(nc) as tc:
        with (
            tc.tile_pool(name="const", bufs=1) as cpool,
            tc.tile_pool(name="work", bufs=3) as wpool,
            tc.tile_pool(name="gat", bufs=4) as gatpool,
            tc.tile_pool(name="ps_att", bufs=2, space="PSUM") as ps_att_pool,
            tc.tile_pool(name="ps_sc", bufs=1, space="PSUM") as ps_sc_pool,
            tc.tile_pool(name="ps_misc", bufs=1, space="PSUM") as ps_misc_pool,
            tc.tile_pool(name="ps_gat", bufs=1, space="PSUM") as ps_gat_pool,
        ):
            # ---- constants into SBUF (P-stage inputs first) ----
            W1a_sb, localT_sb = [], []
            for b in range(BPC):
                localT_sb.append([])
            for kt, (k0, kk) in enumerate(H_T):
                t = cpool.tile([kk, H], f32, tag=f"w1a{kt}", name=f"w1a{kt}")
                nc.sync.dma_start(out=t[:], in_=W1d[k0:k0 + kk, :])
                W1a_sb.append(t)
                for b in range(BPC):
                    lt = cpool.tile([kk, N], f32, tag=f"lT{b}_{kt}",
                                    name=f"lT{b}_{kt}")
                    nc.sync.dma_start(out=lt[:], in_=localT[b, k0:k0 + kk, :])
                    localT_sb[b].append(lt)
            b1rep = cpool.tile([128, H], f32, tag="b1rep", name="b1rep")
            nc.sync.dma_start(out=b1rep[:], in_=b1d[0:1, :].to_broadcast([128, H]))
            b2rep = cpool.tile([128, 1], f32, tag="b2rep", name="b2rep")
            nc.sync.dma_start(out=b2rep[:], in_=b2d[0:1, :].to_broadcast([128, 1]))
            W2c_sb = []
            for ht, (h0, hh) in enumerate(H_T):
                t = cpool.tile([hh, 1], bf16, tag=f"w2c{ht}", name=f"w2c{ht}")
                nc.sync.dma_start(out=t[:], in_=W216d[h0:h0 + hh, :])
                W2c_sb.append(t)
            lnat16_sb = []
            for b in range(BPC):
                t = cpool.tile([N, H], bf16, tag=f"ln{b}", name=f"ln{b}")
                nc.sync.dma_start(out=t[:], in_=lnat16[b * N:(b + 1) * N, :])
                lnat16_sb.append(t)

            # rhs triple-buffers: rows 0..104 = indJ5 (constant), rows
            # 105..115 = per-chunk binary features
            bt3 = []
            for ci in range(4):
                t = cpool.tile([KC, CH], bf16, tag=f"bt{ci}", name=f"bt{ci}")
                nc.sync.dma_start(out=t[0:N + CH_I, :], in_=indJ5d[:, :])
                bt3.append(t)

            # ---- P-stage for both batches up front ----
            Cb_all, Pb16_all = [], []
            for b in range(BPC):
                ps_p = ps_misc_pool.tile([N, H], f32, tag="misc", name=f"psp{b}")
                for kt, (k0, kk) in enumerate(H_T):
                    nc.tensor.matmul(out=ps_p[:], lhsT=localT_sb[b][kt][:],
                                     rhs=W1a_sb[kt][:],
                                     start=(kt == 0), stop=(kt == 2))
                # C buffers (lhsT): rows 0..99 = P (j term); rows 100..104 =
                # per-chunk Pb rows (i term, +b1); rows 105..115 = W1b
                Cb = []
                for ci in range(3):
                    c_t = cpool.tile([KC, H], bf16, tag=f"c{b}_{ci}",
                                     name=f"c{b}_{ci}")
                    nc.vector.tensor_copy(out=c_t[0:N, :], in_=ps_p[:])
                    nc.sync.dma_start(out=c_t[N + CH_I:KC, :], in_=W1b16d[:, :])
                    Cb.append(c_t)
                Pb16 = cpool.tile([N, H], bf16, tag=f"pb{b}", name=f"pb{b}")
                nc.vector.tensor_add(out=Pb16[:], in0=ps_p[:], in1=b1rep[0:N, :])
                Cb_all.append(Cb)
                Pb16_all.append(Pb16)

            oh_sb = cpool.tile([N, CAP], bf16, tag="oh", name="oh")

            def emit_chunks(b):
                Cb = Cb_all[b]
                Pb16 = Pb16_all[b]
                scTflat = cpool.tile([1, N * N], bf16, tag=f"scf{b}",
                                     name=f"scf{b}")
                for ic in range(NCH):
                    i0 = ic * CH_I
                    C = Cb[ic % 3]
                    bt = bt3[ic % 4]
                    # per-chunk dynamic rows
                    nc.gpsimd.dma_start(out=C[N:N + CH_I, :],
                                        in_=Pb16[i0:i0 + CH_I, :])
                    nc.sync.dma_start(out=bt[N + CH_I:KC, :],
                                      in_=binT[b, :, ic * CH:(ic + 1) * CH])
                    ps_sc = ps_sc_pool.tile([1, CH], f32, tag="sc",
                                            name=f"pssc{b}_{ic}")
                    for ht, (h0, hh) in enumerate(H_T):
                        ps_a = ps_att_pool.tile(
                            [hh, CH], f32, tag=f"att{ht}",
                            bufs=(1 if ht == 2 else 2),
                            name=f"psa{b}_{ic}_{ht}")
                        nc.tensor.matmul(out=ps_a[:], lhsT=C[:, h0:h0 + hh],
                                         rhs=bt[:], start=True, stop=True)
                        att16 = wpool.tile([hh, CH], bf16, tag=f"att16_{ht}",
                                           name=f"att16_{b}_{ic}_{ht}")
                        if ht == 2:
                            nc.scalar.activation(att16[:], ps_a[:], Relu)
                        else:
                            nc.vector.tensor_scalar_max(out=att16[:],
                                                        in0=ps_a[:],
                                                        scalar1=0.0)
                        nc.tensor.matmul(out=ps_sc[:], lhsT=W2c_sb[ht][:],
                                         rhs=att16[:],
                                         start=(ht == 0), stop=(ht == 2))
                    # sigmoid + write j-major: scTflat[j*N + i] = score[i,j]
                    out_ap = scTflat[0:1, :].rearrange(
                        "p (j i) -> p i j", j=N)[:, i0:i0 + CH_I, :]
                    nc.scalar.activation(
                        out_ap,
                        ps_sc[:1, :].rearrange("p (i j) -> p i j", i=CH_I),
                        Sigmoid, bias=b2rep[0:1, :1])
                return scTflat

            def emit_g(b, scTflat):
                # partition-scatter: scT[j, i] <- scTflat[j*N + i]
                scT = cpool.tile([N, N], bf16, tag=f"sct{b}", name=f"sct{b}")
                nc.sync.dma_start(
                    out=scT[:],
                    in_=scTflat[0:1, :].rearrange("p (j i) -> p j i", j=N))
                ps_g = ps_gat_pool.tile([N, H], f32, tag="gat", name=f"psg{b}")
                nc.tensor.matmul(out=ps_g[:], lhsT=scT[:], rhs=lnat16_sb[b][:],
                                 start=True, stop=True)
                g16 = cpool.tile([N, H], bf16, tag=f"g16_{b}", name=f"g16_{b}")
                nc.scalar.copy(out=g16[:], in_=ps_g[:])
                return g16

            def emit_gather(b, t_i, rhs_sb, outd, tagp):
                # tile t_i of batch b: rows [b*CAP_B + t_i*128 ...]
                row0 = b * CAP_B + t_i * 128
                sl = slice(row0, row0 + 128)
                ps = ps_gat_pool.tile([128, H], f32, tag="gat",
                                      name=f"p{tagp}{b}_{t_i}")
                nc.tensor.matmul(out=ps[:], lhsT=oh_sb[:, row0:row0 + 128],
                                 rhs=rhs_sb[:], start=True, stop=True)
                go = gatpool.tile([128, H], bf16, tag=f"go{tagp}",
                                  name=f"go{tagp}{b}_{t_i}")
                if t_i % 2 == 0:
                    nc.scalar.copy(out=go[:], in_=ps[:])
                else:
                    nc.vector.tensor_copy(out=go[:], in_=ps[:])
                nc.sync.dma_start(out=outd[sl, :], in_=go[:])

            # schedule: b0 chunks -> lp gathers (PE filler while b0 scores
            # drain) -> G(b0) -> gp gathers for b0 -> b1 chunks -> G(b1) ->
            # gp gathers for b1
            scf0 = emit_chunks(0)
            # one-hot matrix streams in while batch-0 chunks execute
            for q in range(4):
                qs = CAP // 4
                nc.sync.dma_start(out=oh_sb[:, q * qs:(q + 1) * qs],
                                  in_=ohd[:, q * qs:(q + 1) * qs])
            for t_i in range(NT_B):
                emit_gather(0, t_i, lnat16_sb[0], lpd, "l")
            g16_0 = emit_g(0, scf0)
            for t_i in range(NT_B):
                emit_gather(0, t_i, g16_0, gpd, "g")
            scf1 = emit_chunks(1)
            for t_i in range(NT_B):
                emit_gather(1, t_i, lnat16_sb[1], lpd, "l")
            g16_1 = emit_g(1, scf1)
            for t_i in range(NT_B):
                emit_gather(1, t_i, g16_1, gpd, "g")

    nc.compile()
    return nc


def _prep_inputs(local_feats, binary_feats, sparse_idx, W1, b1, W2, b2):
    """Build per-core in_maps + reassembly info. Host-side layout only."""
    import ml_dtypes
    bf = ml_dtypes.bfloat16
    local_feats = np.ascontiguousarray(local_feats, dtype=np.float32)
    binary_feats = np.ascontiguousarray(binary_feats, dtype=np.float32)
    sparse_idx = np.asarray(sparse_idx)
    W1 = np.ascontiguousarray(W1, dtype=np.float32)
    b1 = np.ascontiguousarray(b1, dtype=np.float32).reshape(1, H)
    W2 = np.ascontiguousarray(W2, dtype=np.float32).reshape(H, 1)
    b2 = np.ascontiguousarray(b2, dtype=np.float32).reshape(1, 1)
    W1b16 = W1[H:].astype(bf)
    W216 = W2.astype(bf)

    # indJ5: rows 0..99 select the j term (tiled identity), rows 100..104
    # select the i term (block indicator)
    indJ5 = np.zeros((N + CH_I, CH), dtype=np.float32)
    for s in range(CH_I):
        indJ5[np.arange(N), s * N + np.arange(N)] = 1.0
        indJ5[N + s, s * N:(s + 1) * N] = 1.0
    indJ5 = indJ5.astype(bf)

    bb = sparse_idx[:, 0].astype(np.int64)
    ii = sparse_idx[:, 1].astype(np.int64)
    jj = sparse_idx[:, 2].astype(np.int64)

    in_maps, pos_list = [], []
    for c in range(NCORES):
        oh = np.zeros((N, CAP), dtype=np.float32)
        pos_c = []
        for b in range(BPC):
            gb = c * BPC + b
            pos = np.nonzero(bb == gb)[0]
            assert len(pos) <= CAP_B, \
                f"core {c} batch {b}: {len(pos)} entries > CAP_B={CAP_B}"
            cols = b * CAP_B + np.arange(len(pos))
            np.add.at(oh, (ii[pos], cols), 1.0)
            np.add.at(oh, (jj[pos], cols), 1.0)
            pos_c.append(pos)
        oh = oh.astype(bf)
        sl = slice(c * BPC, c * BPC + BPC)
        lnat_c = np.ascontiguousarray(local_feats[sl].reshape(BPC * N, H))
        in_maps.append({
            "localT": np.ascontiguousarray(local_feats[sl].transpose(0, 2, 1)),
            "lnat16": lnat_c.astype(bf),
            "binT": np.ascontiguousarray(
                binary_feats[sl].transpose(0, 3, 1, 2).reshape(
                    BPC, BIN, N * N)).astype(bf),
            "W1": W1, "W1b16": W1b16, "W216": W216,
            "b1": b1, "b2": b2,
            "indJ5": indJ5, "oh": oh,
        })
        pos_list.append(pos_c)
    return in_maps, pos_list


def _run(in_maps, trace=False):
    from concourse.bass_utils import run_bass_kernel_spmd
    if "nc" not in _CACHE:
        _CACHE["nc"] = _build_nc()
    nc = _CACHE["nc"]
    res = run_bass_kernel_spmd(nc, in_maps, core_ids=list(range(NCORES)),
                               trace=trace)
    return res


def kernel(local_feats, binary_feats, sparse_idx, W1, b1, W2, b2):
    in_maps, pos_list = _prep_inputs(local_feats, binary_feats, sparse_idx,
                                     W1, b1, W2, b2)
    res = _run(in_maps)
    E = sparse_idx.shape[0]
    lp_full = np.zeros((E, H), dtype=np.float32)
    gp_full = np.zeros((E, H), dtype=np.float32)
    for c in range(NCORES):
        for b in range(BPC):
            pos = pos_list[c][b]
            r0 = b * CAP_B
            lp_full[pos] = res.results[c]["lp"][r0:r0 + len(pos)].astype(
                np.float32)
            gp_full[pos] = res.results[c]["gp"][r0:r0 + len(pos)].astype(
                np.float32)
    return (lp_full, gp_full)


# revision 35
# speedup vs baseline: 1.0467x; 1.0467x over previous
"""Trainium2 Bass kernel for nn_Attention_14370960572643 (gnn_message_passing).

Math (per batch b):
  local_pair[b,i,j,:] = local[b,i,:] + local[b,j,:]
  att  = relu(concat(local_pair, binary) @ W1 + b1)        [B,N,N,H]
  score = sigmoid(att @ W2 + b2)                            [B,N,N,1]
  G[b,i,:] = sum_j local[b,j,:] * score[b,i,j]              [B,N,H]
  outputs (E sparse pairs): lp[e] = local[bb,ii]+local[bb,jj]
                            gp[e] = G[bb,ii]+G[bb,jj]

Key tricks:
  * local_pair @ W1a = P[b,i,:] + P[b,j,:] with P = local @ W1[:H] — the
    [B*N*N, 311] einsum collapses into ONE K=116 matmul per h-tile:
    lhsT rows 0..99 hold P (j term), rows 100..104 hold P+b1 for the
    chunk's 5 i values, rows 105..115 hold W1b; the rhs pairs those with
    a constant 0/1 indicator (rows 0..104) and the chunk's binary
    features (rows 105..115). PSUM gets Q + P_j + P_i + b1 in one pass;
    the epilogue is a single relu (split across DVE and ACT).
  * sparse gathers lp/gp are one-hot matmuls (the ii+jj add comes free);
    sparse entries are grouped by batch so each 128-row tile needs one
    matmul, and the batch-0 half of gp runs overlapped with batch-1
    compute.
  * scoreT ([j,i] layout for the G matmul) is produced by a strided
    sigmoid write plus one partition-scatter DMA per batch.

Sharding: data-parallel over B, 2 batches per core, 8 cores.
sparse_idx entries are routed to the core owning their batch.
"""

import numpy as np

B, N, H, BIN = 16, 100, 300, 11
HB = H + BIN  # 311
KC = 116                   # combined contraction: 100 P + 5 Pb + 11 W1b
NCORES = 8
BPC = B // NCORES          # batches per core
CAP_B = 1536               # padded sparse entries per (core, batch)
NT_B = CAP_B // 128        # 12 gather tiles per batch
CAP = CAP_B * BPC          # 3072 per core
NT = NT_B * BPC
CH_I = 5                   # i values per chunk
CH = CH_I * N              # 500 rows per chunk
NCH = N // CH_I            # 20 chunks per batch
H_T = [(0, 128), (128, 128), (256, 44)]   # h tiles (also used for k over H)

_CACHE = {}


def _build_nc():
    import concourse.bass as bass
    import concourse.mybir as mybir
    import concourse.tile as tile
    from concourse import bacc

    dt = mybir.dt
    f32 = dt.float32
    bf16 = dt.bfloat16

    nc = bacc.Bacc("TRN2", target_bir_lowering=False, debug=False,
                   num_devices=NCORES)

    # ---- dram parameters (per-core shards) ----
    localT = nc.dram_tensor("localT", [BPC, H, N], bf16, kind="ExternalInput").ap()
    lnat16 = nc.dram_tensor("lnat16", [BPC * N, H], bf16, kind="ExternalInput").ap()
    binT = nc.dram_tensor("binT", [BPC, BIN, N * N], bf16, kind="ExternalInput").ap()
    W1d = nc.dram_tensor("W1", [HB, H], bf16, kind="ExternalInput").ap()
    W1b16d = nc.dram_tensor("W1b16", [BIN, H], bf16, kind="ExternalInput").ap()
    W216d = nc.dram_tensor("W216", [H, 1], bf16, kind="ExternalInput").ap()
    b1d = nc.dram_tensor("b1", [1, H], f32, kind="ExternalInput").ap()
    b2d = nc.dram_tensor("b2", [1, 1], f32, kind="ExternalInput").ap()
    indJ5d = nc.dram_tensor("indJ5", [N + CH_I, CH], bf16,
                            kind="ExternalInput").ap()
    ohd = nc.dram_tensor("oh", [N, CAP], bf16, kind="ExternalInput").ap()
    lpd = nc.dram_tensor("lp", [CAP, H], bf16, kind="ExternalOutput").ap()
    gpd = nc.dram_tensor("gp", [CAP, H], bf16, kind="ExternalOutput").ap()

    Relu = mybir.ActivationFunctionType.Relu
    Sigmoid = mybir.ActivationFunctionType.Sigmoid

    with tile.TileContext(nc) as tc:
        with (
            tc.tile_pool(name="const", bufs=1) as cpool,
            tc.tile_pool(name="work", bufs=3) as wpool,
            tc.tile_pool(name="gat", bufs=4) as gatpool,
            tc.tile_pool(name="ps_att", bufs=2, space="PSUM") as ps_att_pool,
            tc.tile_pool(name="ps_sc", bufs=1, space="PSUM") as ps_sc_pool,
            tc.tile_pool(name="ps_misc", bufs=1, space="PSUM") as ps_misc_pool,
            tc.tile_pool(name="ps_gat", bufs=1, space="PSUM") as ps_gat_pool,
        ):
            # ---- constants into SBUF (P-stage inputs first) ----
            W1a_sb, localT_sb = [], []
            for b in range(BPC):
                localT_sb.append([])
            for kt, (k0, kk) in enumerate(H_T):
                t = cpool.tile([kk, H], bf16, tag=f"w1a{kt}", name=f"w1a{kt}")
                nc.sync.dma_start(out=t[:], in_=W1d[k0:k0 + kk, :])
                W1a_sb.append(t)
                for b in range(BPC):
                    lt = cpool.tile([kk, N], bf16, tag=f"lT{b}_{kt}",
                                    name=f"lT{b}_{kt}")
                    nc.sync.dma_start(out=lt[:], in_=localT[b, k0:k0 + kk, :])
                    localT_sb[b].append(lt)
            b1rep = cpool.tile([128, H], f32, tag="b1rep", name="b1rep")
            nc.sync.dma_start(out=b1rep[:], in_=b1d[0:1, :].to_broadcast([128, H]))
            b2rep = cpool.tile([128, 1], f32, tag="b2rep", name="b2rep")
            nc.sync.dma_start(out=b2rep[:], in_=b2d[0:1, :].to_broadcast([128, 1]))
            # rhs triple-buffers: rows 0..104 = indJ5 (constant), rows
            # 105..115 = per-chunk binary features
            bt3 = []
            for ci in range(4):
                t = cpool.tile([KC, CH], bf16, tag=f"bt{ci}", name=f"bt{ci}")
                nc.sync.dma_start(out=t[0:N + CH_I, :], in_=indJ5d[:, :])
                bt3.append(t)

            # ---- P-stage for both batches up front ----
            Cb_all, Pb16_all = [], []
            for b in range(BPC):
                ps_p = ps_misc_pool.tile([N, H], f32, tag="misc", name=f"psp{b}")
                for kt, (k0, kk) in enumerate(H_T):
                    nc.tensor.matmul(out=ps_p[:], lhsT=localT_sb[b][kt][:],
                                     rhs=W1a_sb[kt][:],
                                     start=(kt == 0), stop=(kt == 2))
                # C buffers (lhsT): rows 0..99 = P (j term); rows 100..104 =
                # per-chunk Pb rows (i term, +b1); rows 105..115 = W1b
                Cb = []
                for ci in range(3):
                    c_t = cpool.tile([KC, H], bf16, tag=f"c{b}_{ci}",
                                     name=f"c{b}_{ci}")
                    nc.vector.tensor_copy(out=c_t[0:N, :], in_=ps_p[:])
                    nc.sync.dma_start(out=c_t[N + CH_I:KC, :], in_=W1b16d[:, :])
                    Cb.append(c_t)
                Pb16 = cpool.tile([N, H], bf16, tag=f"pb{b}", name=f"pb{b}")
                nc.vector.tensor_add(out=Pb16[:], in0=ps_p[:], in1=b1rep[0:N, :])
                Cb_all.append(Cb)
                Pb16_all.append(Pb16)

            W2c_sb = []
            for ht, (h0, hh) in enumerate(H_T):
                t = cpool.tile([hh, 1], bf16, tag=f"w2c{ht}", name=f"w2c{ht}")
                nc.sync.dma_start(out=t[:], in_=W216d[h0:h0 + hh, :])
                W2c_sb.append(t)
            lnat16_sb = []
            for b in range(BPC):
                t = cpool.tile([N, H], bf16, tag=f"ln{b}", name=f"ln{b}")
                nc.sync.dma_start(out=t[:], in_=lnat16[b * N:(b + 1) * N, :])
                lnat16_sb.append(t)

            oh_sb = cpool.tile([N, CAP], bf16, tag="oh", name="oh")

            def emit_chunks(b, fillers=()):
                fillers = list(fillers)
                Cb = Cb_all[b]
                Pb16 = Pb16_all[b]
                scTflat = cpool.tile([1, N * N], bf16, tag=f"scf{b}",
                                     name=f"scf{b}")
                for ic in range(NCH):
                    i0 = ic * CH_I
                    C = Cb[ic % 3]
                    bt = bt3[ic % 4]
                    # per-chunk dynamic rows
                    nc.gpsimd.dma_start(out=C[N:N + CH_I, :],
                                        in_=Pb16[i0:i0 + CH_I, :])
                    nc.sync.dma_start(out=bt[N + CH_I:KC, :],
                                      in_=binT[b, :, ic * CH:(ic + 1) * CH])
                    ps_sc = ps_sc_pool.tile([1, CH], f32, tag="sc",
                                            name=f"pssc{b}_{ic}")
                    for ht, (h0, hh) in enumerate(H_T):
                        ps_a = ps_att_pool.tile(
                            [hh, CH], f32, tag=f"att{ht}",
                            bufs=(1 if ht == 2 else 2),
                            name=f"psa{b}_{ic}_{ht}")
                        nc.tensor.matmul(out=ps_a[:], lhsT=C[:, h0:h0 + hh],
                                         rhs=bt[:], start=True, stop=True)
                        att16 = wpool.tile([hh, CH], bf16, tag=f"att16_{ht}",
                                           name=f"att16_{b}_{ic}_{ht}")
                        if ht == 2:
                            nc.scalar.activation(att16[:], ps_a[:], Relu)
                        else:
                            nc.vector.tensor_scalar_max(out=att16[:],
                                                        in0=ps_a[:],
                                                        scalar1=0.0)
                        nc.tensor.matmul(out=ps_sc[:], lhsT=W2c_sb[ht][:],
                                         rhs=att16[:],
                                         start=(ht == 0), stop=(ht == 2))
                    # sigmoid + write j-major: scTflat[j*N + i] = score[i,j]
                    out_ap = scTflat[0:1, :].rearrange(
                        "p (j i) -> p i j", j=N)[:, i0:i0 + CH_I, :]
                    nc.scalar.activation(
                        out_ap,
                        ps_sc[:1, :].rearrange("p (i j) -> p i j", i=CH_I),
                        Sigmoid, bias=b2rep[0:1, :1])
                    if ic >= 6 and fillers:
                        fillers.pop(0)()
                return scTflat

            def emit_g(b, scTflat):
                # partition-scatter: scT[j, i] <- scTflat[j*N + i]
                scT = cpool.tile([N, N], bf16, tag=f"sct{b}", name=f"sct{b}")
                nc.sync.dma_start(
                    out=scT[:],
                    in_=scTflat[0:1, :].rearrange("p (j i) -> p j i", j=N))
                ps_g = ps_gat_pool.tile([N, H], f32, tag="gat", name=f"psg{b}")
                nc.tensor.matmul(out=ps_g[:], lhsT=scT[:], rhs=lnat16_sb[b][:],
                                 start=True, stop=True)
                g16 = cpool.tile([N, H], bf16, tag=f"g16_{b}", name=f"g16_{b}")
                nc.scalar.copy(out=g16[:], in_=ps_g[:])
                return g16

            def emit_gather(b, t_i, rhs_sb, outd, tagp):
                # tile t_i of batch b: rows [b*CAP_B + t_i*128 ...]
                row0 = b * CAP_B + t_i * 128
                sl = slice(row0, row0 + 128)
                pool = ps_gat_pool if t_i % 2 == 0 else ps_misc_pool
                ps = pool.tile([128, H], f32,
                               tag=("gat" if t_i % 2 == 0 else "misc"),
                               name=f"p{tagp}{b}_{t_i}")
                nc.tensor.matmul(out=ps[:], lhsT=oh_sb[:, row0:row0 + 128],
                                 rhs=rhs_sb[:], start=True, stop=True)
                go = gatpool.tile([128, H], bf16, tag=f"go{tagp}",
                                  name=f"go{tagp}{b}_{t_i}")
                if t_i % 2 == 0:
                    nc.scalar.copy(out=go[:], in_=ps[:])
                else:
                    nc.vector.tensor_copy(out=go[:], in_=ps[:])
                nc.sync.dma_start(out=outd[sl, :], in_=go[:])

            # schedule: b0 chunks -> lp gathers (PE filler while b0 scores
            # drain) -> G(b0) -> gp gathers for b0 -> b1 chunks -> G(b1) ->
            # gp gathers for b1
            def lp_filler(b, t_i):
                return lambda: emit_gather(b, t_i, lnat16_sb[b], lpd, "l")

            # one-hot matrix loads must precede the lp fillers that read it
            for q in range(4):
                qs = CAP // 4
                nc.sync.dma_start(out=oh_sb[:, q * qs:(q + 1) * qs],
                                  in_=ohd[:, q * qs:(q + 1) * qs])
            scf0 = emit_chunks(
                0, [lp_filler(0, t) for t in range(NT_B)])
            g16_0 = emit_g(0, scf0)
            for t_i in range(NT_B):
                emit_gather(0, t_i, g16_0, gpd, "g")
            scf1 = emit_chunks(
                1, [lp_filler(1, t) for t in range(NT_B)])
            g16_1 = emit_g(1, scf1)
            for t_i in range(NT_B):
                emit_gather(1, t_i, g16_1, gpd, "g")

    nc.compile()
    return nc


def _prep_inputs(local_feats, binary_feats, sparse_idx, W1, b1, W2, b2):
    """Build per-core in_maps + reassembly info. Host-side layout only."""
    import ml_dtypes
    bf = ml_dtypes.bfloat16
    local_feats = np.ascontiguousarray(local_feats, dtype=np.float32)
    binary_feats = np.ascontiguousarray(binary_feats, dtype=np.float32)
    sparse_idx = np.asarray(sparse_idx)
    W1 = np.ascontiguousarray(W1, dtype=np.float32)
    b1 = np.ascontiguousarray(b1, dtype=np.float32).reshape(1, H)
    W2 = np.ascontiguousarray(W2, dtype=np.float32).reshape(H, 1)
    b2 = np.ascontiguousarray(b2, dtype=np.float32).reshape(1, 1)
    W1b16 = W1[H:].astype(bf)
    W216 = W2.astype(bf)

    # indJ5: rows 0..99 select the j term (tiled identity), rows 100..104
    # select the i term (block indicator)
    indJ5 = np.zeros((N + CH_I, CH), dtype=np.float32)
    for s in range(CH_I):
        indJ5[np.arange(N), s * N + np.arange(N)] = 1.0
        indJ5[N + s, s * N:(s + 1) * N] = 1.0
    indJ5 = indJ5.astype(bf)

    bb = sparse_idx[:, 0].astype(np.int64)
    ii = sparse_idx[:, 1].astype(np.int64)
    jj = sparse_idx[:, 2].astype(np.int64)

    in_maps, pos_list = [], []
    for c in range(NCORES):
        oh = np.zeros((N, CAP), dtype=np.float32)
        pos_c = []
        for b in range(BPC):
            gb = c * BPC + b
            pos = np.nonzero(bb == gb)[0]
            assert len(pos) <= CAP_B, \
                f"core {c} batch {b}: {len(pos)} entries > CAP_B={CAP_B}"
            cols = b * CAP_B + np.arange(len(pos))
            np.add.at(oh, (ii[pos], cols), 1.0)
            np.add.at(oh, (jj[pos], cols), 1.0)
            pos_c.append(pos)
        oh = oh.astype(bf)
        sl = slice(c * BPC, c * BPC + BPC)
        lnat_c = np.ascontiguousarray(local_feats[sl].reshape(BPC * N, H))
        in_maps.append({
            "localT": np.ascontiguousarray(
                local_feats[sl].transpose(0, 2, 1)).astype(bf),
            "lnat16": lnat_c.astype(bf),
            "binT": np.ascontiguousarray(
                binary_feats[sl].transpose(0, 3, 1, 2).reshape(
                    BPC, BIN, N * N)).astype(bf),
            "W1": W1.astype(bf), "W1b16": W1b16, "W216": W216,
            "b1": b1, "b2": b2,
            "indJ5": indJ5, "oh": oh,
        })
        pos_list.append(pos_c)
    return in_maps, pos_list


def _run(in_maps, trace=False):
    from concourse.bass_utils import run_bass_kernel_spmd
    if "nc" not in _CACHE:
        _CACHE["nc"] = _build_nc()
    nc = _CACHE["nc"]
    res = run_bass_kernel_spmd(nc, in_maps, core_ids=list(range(NCORES)),
                               trace=trace)
    return res


def kernel(local_feats, binary_feats, sparse_idx, W1, b1, W2, b2):
    in_maps, pos_list = _prep_inputs(local_feats, binary_feats, sparse_idx,
                                     W1, b1, W2, b2)
    res = _run(in_maps)
    E = sparse_idx.shape[0]
    lp_full = np.zeros((E, H), dtype=np.float32)
    gp_full = np.zeros((E, H), dtype=np.float32)
    for c in range(NCORES):
        for b in range(BPC):
            pos = pos_list[c][b]
            r0 = b * CAP_B
            lp_full[pos] = res.results[c]["lp"][r0:r0 + len(pos)].astype(
                np.float32)
            gp_full[pos] = res.results[c]["gp"][r0:r0 + len(pos)].astype(
                np.float32)
    return (lp_full, gp_full)


# revision 36
# speedup vs baseline: 1.0602x; 1.0129x over previous
"""Trainium2 Bass kernel for nn_Attention_14370960572643 (gnn_message_passing).

Math (per batch b):
  local_pair[b,i,j,:] = local[b,i,:] + local[b,j,:]
  att  = relu(concat(local_pair, binary) @ W1 + b1)        [B,N,N,H]
  score = sigmoid(att @ W2 + b2)                            [B,N,N,1]
  G[b,i,:] = sum_j local[b,j,:] * score[b,i,j]              [B,N,H]
  outputs (E sparse pairs): lp[e] = local[bb,ii]+local[bb,jj]
                            gp[e] = G[bb,ii]+G[bb,jj]

Key tricks:
  * local_pair @ W1a = P[b,i,:] + P[b,j,:] with P = local @ W1[:H] — the
    [B*N*N, 311] einsum collapses into ONE K=116 matmul per h-tile:
    lhsT rows 0..99 hold P (j term), rows 100..104 hold P+b1 for the
    chunk's 5 i values, rows 105..115 hold W1b; the rhs pairs those with
    a constant 0/1 indicator (rows 0..104) and the chunk's binary
    features (rows 105..115). PSUM gets Q + P_j + P_i + b1 in one pass;
    the epilogue is a single relu (split across DVE and ACT).
  * sparse gathers lp/gp are one-hot matmuls (the ii+jj add comes free);
    sparse entries are grouped by batch so each 128-row tile needs one
    matmul, and the batch-0 half of gp runs overlapped with batch-1
    compute.
  * scoreT ([j,i] layout for the G matmul) is produced by a strided
    sigmoid write plus one partition-scatter DMA per batch.

Sharding: data-parallel over B, 2 batches per core, 8 cores.
sparse_idx entries are routed to the core owning their batch.
"""

import numpy as np

B, N, H, BIN = 16, 100, 300, 11
HB = H + BIN  # 311
KC = 116                   # combined contraction: 100 P + 5 Pb + 11 W1b
NCORES = 8
BPC = B // NCORES          # batches per core
CAP_B = 1536               # padded sparse entries per (core, batch)
NT_B = CAP_B // 128        # 12 gather tiles per batch
CAP = CAP_B * BPC          # 3072 per core
NT = NT_B * BPC
CH_I = 5                   # i values per chunk
CH = CH_I * N              # 500 rows per chunk
NCH = N // CH_I            # 20 chunks per batch
H_T = [(0, 128), (128, 128), (256, 44)]   # h tiles (also used for k over H)

_CACHE = {}


def _build_nc():
    import concourse.bass as bass
    import concourse.mybir as mybir
    import concourse.tile as tile
    from concourse import bacc

    dt = mybir.dt
    f32 = dt.float32
    bf16 = dt.bfloat16

    nc = bacc.Bacc("TRN2", target_bir_lowering=False, debug=False,
                   num_devices=NCORES)

    # ---- dram parameters (per-core shards) ----
    localT = nc.dram_tensor("localT", [BPC, H, N], bf16, kind="ExternalInput").ap()
    lnat16 = nc.dram_tensor("lnat16", [BPC * N, H], bf16, kind="ExternalInput").ap()
    binT = nc.dram_tensor("binT", [BPC, BIN, N * N], bf16, kind="ExternalInput").ap()
    W1d = nc.dram_tensor("W1", [HB, H], bf16, kind="ExternalInput").ap()
    W1b16d = nc.dram_tensor("W1b16", [BIN, H], bf16, kind="ExternalInput").ap()
    W216d = nc.dram_tensor("W216", [H, 1], bf16, kind="ExternalInput").ap()
    b1d = nc.dram_tensor("b1", [1, H], f32, kind="ExternalInput").ap()
    b2d = nc.dram_tensor("b2", [1, 1], f32, kind="ExternalInput").ap()
    indJ5d = nc.dram_tensor("indJ5", [N + CH_I, CH], bf16,
                            kind="ExternalInput").ap()
    ohd = nc.dram_tensor("oh", [N, CAP], bf16, kind="ExternalInput").ap()
    lpd = nc.dram_tensor("lp", [CAP, H], bf16, kind="ExternalOutput").ap()
    gpd = nc.dram_tensor("gp", [CAP, H], bf16, kind="ExternalOutput").ap()

    Relu = mybir.ActivationFunctionType.Relu
    Sigmoid = mybir.ActivationFunctionType.Sigmoid

    with tile.TileContext(nc) as tc:
        with (
            tc.tile_pool(name="const", bufs=1) as cpool,
            tc.tile_pool(name="work", bufs=3) as wpool,
            tc.tile_pool(name="gat", bufs=4) as gatpool,
            tc.tile_pool(name="ps_att", bufs=2, space="PSUM") as ps_att_pool,
            tc.tile_pool(name="ps_sc", bufs=1, space="PSUM") as ps_sc_pool,
            tc.tile_pool(name="ps_misc", bufs=1, space="PSUM") as ps_misc_pool,
            tc.tile_pool(name="ps_gat", bufs=1, space="PSUM") as ps_gat_pool,
        ):
            # ---- constants into SBUF (P-stage inputs first) ----
            W1a_sb, localT_sb = [], []
            for b in range(BPC):
                localT_sb.append([])
            for kt, (k0, kk) in enumerate(H_T):
                t = cpool.tile([kk, H], bf16, tag=f"w1a{kt}", name=f"w1a{kt}")
                nc.sync.dma_start(out=t[:], in_=W1d[k0:k0 + kk, :])
                W1a_sb.append(t)
                for b in range(BPC):
                    lt = cpool.tile([kk, N], bf16, tag=f"lT{b}_{kt}",
                                    name=f"lT{b}_{kt}")
                    nc.sync.dma_start(out=lt[:], in_=localT[b, k0:k0 + kk, :])
                    localT_sb[b].append(lt)
            b1rep = cpool.tile([128, H], f32, tag="b1rep", name="b1rep")
            nc.sync.dma_start(out=b1rep[:], in_=b1d[0:1, :].to_broadcast([128, H]))
            b2rep = cpool.tile([128, 1], f32, tag="b2rep", name="b2rep")
            nc.sync.dma_start(out=b2rep[:], in_=b2d[0:1, :].to_broadcast([128, 1]))
            # rhs double-buffers covering TWO chunks each: rows 0..104 =
            # indJ5 (constant, both halves), rows 105..115 = binary features
            bt3 = []
            for ci in range(2):
                t = cpool.tile([KC, 2 * CH], bf16, tag=f"bt{ci}", name=f"bt{ci}")
                nc.sync.dma_start(out=t[0:N + CH_I, 0:CH], in_=indJ5d[:, :])
                nc.sync.dma_start(out=t[0:N + CH_I, CH:2 * CH], in_=indJ5d[:, :])
                bt3.append(t)

            # ---- P-stage for both batches up front ----
            Cb_all, Pb16_all = [], []
            for b in range(BPC):
                ps_p = ps_misc_pool.tile([N, H], f32, tag="misc", name=f"psp{b}")
                for kt, (k0, kk) in enumerate(H_T):
                    nc.tensor.matmul(out=ps_p[:], lhsT=localT_sb[b][kt][:],
                                     rhs=W1a_sb[kt][:],
                                     start=(kt == 0), stop=(kt == 2))
                # C buffers (lhsT): rows 0..99 = P (j term); rows 100..104 =
                # per-chunk Pb rows (i term, +b1); rows 105..115 = W1b
                Cb = []
                for ci in range(3):
                    c_t = cpool.tile([KC, H], bf16, tag=f"c{b}_{ci}",
                                     name=f"c{b}_{ci}")
                    nc.vector.tensor_copy(out=c_t[0:N, :], in_=ps_p[:])
                    nc.sync.dma_start(out=c_t[N + CH_I:KC, :], in_=W1b16d[:, :])
                    Cb.append(c_t)
                Pb16 = cpool.tile([N, H], bf16, tag=f"pb{b}", name=f"pb{b}")
                nc.vector.tensor_add(out=Pb16[:], in0=ps_p[:], in1=b1rep[0:N, :])
                Cb_all.append(Cb)
                Pb16_all.append(Pb16)

            W2c_sb = []
            for ht, (h0, hh) in enumerate(H_T):
                t = cpool.tile([hh, 1], bf16, tag=f"w2c{ht}", name=f"w2c{ht}")
                nc.sync.dma_start(out=t[:], in_=W216d[h0:h0 + hh, :])
                W2c_sb.append(t)
            lnat16_sb = []
            for b in range(BPC):
                t = cpool.tile([N, H], bf16, tag=f"ln{b}", name=f"ln{b}")
                nc.sync.dma_start(out=t[:], in_=lnat16[b * N:(b + 1) * N, :])
                lnat16_sb.append(t)

            oh_sb = cpool.tile([N, CAP], bf16, tag="oh", name="oh")

            def emit_chunks(b, fillers=()):
                fillers = list(fillers)
                Cb = Cb_all[b]
                Pb16 = Pb16_all[b]
                scTflat = cpool.tile([1, N * N], bf16, tag=f"scf{b}",
                                     name=f"scf{b}")
                for ic in range(NCH):
                    i0 = ic * CH_I
                    C = Cb[ic % 3]
                    bt = bt3[(ic // 2) % 2]
                    bts = bt[:, (ic % 2) * CH:(ic % 2 + 1) * CH]
                    # per-chunk dynamic rows
                    nc.gpsimd.dma_start(out=C[N:N + CH_I, :],
                                        in_=Pb16[i0:i0 + CH_I, :])
                    if ic % 2 == 0:
                        nc.sync.dma_start(
                            out=bt[N + CH_I:KC, :],
                            in_=binT[b, :, ic * CH:(ic + 2) * CH])
                    ps_sc = ps_sc_pool.tile([1, CH], f32, tag="sc",
                                            name=f"pssc{b}_{ic}")
                    for ht, (h0, hh) in enumerate(H_T):
                        ps_a = ps_att_pool.tile(
                            [hh, CH], f32, tag=f"att{ht}",
                            bufs=(1 if ht == 2 else 2),
                            name=f"psa{b}_{ic}_{ht}")
                        nc.tensor.matmul(out=ps_a[:], lhsT=C[:, h0:h0 + hh],
                                         rhs=bts, start=True, stop=True)
                        att16 = wpool.tile([hh, CH], bf16, tag=f"att16_{ht}",
                                           name=f"att16_{b}_{ic}_{ht}")
                        if ht == 2:
                            nc.scalar.activation(att16[:], ps_a[:], Relu)
                        else:
                            nc.vector.tensor_scalar_max(out=att16[:],
                                                        in0=ps_a[:],
                                                        scalar1=0.0)
                        nc.tensor.matmul(out=ps_sc[:], lhsT=W2c_sb[ht][:],
                                         rhs=att16[:],
                                         start=(ht == 0), stop=(ht == 2))
                    # sigmoid + write j-major: scTflat[j*N + i] = score[i,j]
                    out_ap = scTflat[0:1, :].rearrange(
                        "p (j i) -> p i j", j=N)[:, i0:i0 + CH_I, :]
                    nc.scalar.activation(
                        out_ap,
                        ps_sc[:1, :].rearrange("p (i j) -> p i j", i=CH_I),
                        Sigmoid, bias=b2rep[0:1, :1])
                    if ic >= 6 and fillers:
                        fillers.pop(0)()
                return scTflat

            def emit_g(b, scTflat):
                # partition-scatter: scT[j, i] <- scTflat[j*N + i]
                scT = cpool.tile([N, N], bf16, tag=f"sct{b}", name=f"sct{b}")
                nc.sync.dma_start(
                    out=scT[:],
                    in_=scTflat[0:1, :].rearrange("p (j i) -> p j i", j=N))
                ps_g = ps_gat_pool.tile([N, H], f32, tag="gat", name=f"psg{b}")
                nc.tensor.matmul(out=ps_g[:], lhsT=scT[:], rhs=lnat16_sb[b][:],
                                 start=True, stop=True)
                g16 = cpool.tile([N, H], bf16, tag=f"g16_{b}", name=f"g16_{b}")
                nc.scalar.copy(out=g16[:], in_=ps_g[:])
                return g16

            def emit_gather(b, t_i, rhs_sb, outd, tagp):
                # tile t_i of batch b: rows [b*CAP_B + t_i*128 ...]
                row0 = b * CAP_B + t_i * 128
                sl = slice(row0, row0 + 128)
                pool = ps_gat_pool if t_i % 2 == 0 else ps_misc_pool
                ps = pool.tile([128, H], f32,
                               tag=("gat" if t_i % 2 == 0 else "misc"),
                               name=f"p{tagp}{b}_{t_i}")
                nc.tensor.matmul(out=ps[:], lhsT=oh_sb[:, row0:row0 + 128],
                                 rhs=rhs_sb[:], start=True, stop=True)
                go = gatpool.tile([128, H], bf16, tag=f"go{tagp}",
                                  name=f"go{tagp}{b}_{t_i}")
                if t_i % 2 == 0:
                    nc.scalar.copy(out=go[:], in_=ps[:])
                else:
                    nc.vector.tensor_copy(out=go[:], in_=ps[:])
                nc.sync.dma_start(out=outd[sl, :], in_=go[:])

            # schedule: b0 chunks -> lp gathers (PE filler while b0 scores
            # drain) -> G(b0) -> gp gathers for b0 -> b1 chunks -> G(b1) ->
            # gp gathers for b1
            def lp_filler(b, t_i):
                return lambda: emit_gather(b, t_i, lnat16_sb[b], lpd, "l")

            # one-hot matrix loads must precede the lp fillers that read it
            for q in range(4):
                qs = CAP // 4
                nc.sync.dma_start(out=oh_sb[:, q * qs:(q + 1) * qs],
                                  in_=ohd[:, q * qs:(q + 1) * qs])
            scf0 = emit_chunks(
                0, [lp_filler(0, t) for t in range(NT_B)])
            g16_0 = emit_g(0, scf0)
            for t_i in range(NT_B):
                emit_gather(0, t_i, g16_0, gpd, "g")
            scf1 = emit_chunks(
                1, [lp_filler(1, t) for t in range(NT_B)])
            g16_1 = emit_g(1, scf1)
            for t_i in range(NT_B):
                emit_gather(1, t_i, g16_1, gpd, "g")

    nc.compile()
    return nc


def _prep_inputs(local_feats, binary_feats, sparse_idx, W1, b1, W2, b2):
    """Build per-core in_maps + reassembly info. Host-side layout only."""
    import ml_dtypes
    bf = ml_dtypes.bfloat16
    local_feats = np.ascontiguousarray(local_feats, dtype=np.float32)
    binary_feats = np.ascontiguousarray(binary_feats, dtype=np.float32)
    sparse_idx = np.asarray(sparse_idx)
    W1 = np.ascontiguousarray(W1, dtype=np.float32)
    b1 = np.ascontiguousarray(b1, dtype=np.float32).reshape(1, H)
    W2 = np.ascontiguousarray(W2, dtype=np.float32).reshape(H, 1)
    b2 = np.ascontiguousarray(b2, dtype=np.float32).reshape(1, 1)
    W1b16 = W1[H:].astype(bf)
    W216 = W2.astype(bf)

    # indJ5: rows 0..99 select the j term (tiled identity), rows 100..104
    # select the i term (block indicator)
    indJ5 = np.zeros((N + CH_I, CH), dtype=np.float32)
    for s in range(CH_I):
        indJ5[np.arange(N), s * N + np.arange(N)] = 1.0
        indJ5[N + s, s * N:(s + 1) * N] = 1.0
    indJ5 = indJ5.astype(bf)

    bb = sparse_idx[:, 0].astype(np.int64)
    ii = sparse_idx[:, 1].astype(np.int64)
    jj = sparse_idx[:, 2].astype(np.int64)

    in_maps, pos_list = [], []
    for c in range(NCORES):
        oh = np.zeros((N, CAP), dtype=np.float32)
        pos_c = []
        for b in range(BPC):
            gb = c * BPC + b
            pos = np.nonzero(bb == gb)[0]
            assert len(pos) <= CAP_B, \
                f"core {c} batch {b}: {len(pos)} entries > CAP_B={CAP_B}"
            cols = b * CAP_B + np.arange(len(pos))
            np.add.at(oh, (ii[pos], cols), 1.0)
            np.add.at(oh, (jj[pos], cols), 1.0)
            pos_c.append(pos)
        oh = oh.astype(bf)
        sl = slice(c * BPC, c * BPC + BPC)
        lnat_c = np.ascontiguousarray(local_feats[sl].reshape(BPC * N, H))
        in_maps.append({
            "localT": np.ascontiguousarray(
                local_feats[sl].transpose(0, 2, 1)).astype(bf),
            "lnat16": lnat_c.astype(bf),
            "binT": np.ascontiguousarray(
                binary_feats[sl].transpose(0, 3, 1, 2).reshape(
                    BPC, BIN, N * N)).astype(bf),
            "W1": W1.astype(bf), "W1b16": W1b16, "W216": W216,
            "b1": b1, "b2": b2,
            "indJ5": indJ5, "oh": oh,
        })
        pos_list.append(pos_c)
    return in_maps, pos_list


def _run(in_maps, trace=False):
    from concourse.bass_utils import run_bass_kernel_spmd
    if "nc" not in _CACHE:
        _CACHE["nc"] = _build_nc()
    nc = _CACHE["nc"]
    res = run_bass_kernel_spmd(nc, in_maps, core_ids=list(range(NCORES)),
                               trace=trace)
    return res


def kernel(local_feats, binary_feats, sparse_idx, W1, b1, W2, b2):
    in_maps, pos_list = _prep_inputs(local_feats, binary_feats, sparse_idx,
                                     W1, b1, W2, b2)
    res = _run(in_maps)
    E = sparse_idx.shape[0]
    lp_full = np.zeros((E, H), dtype=np.float32)
    gp_full = np.zeros((E, H), dtype=np.float32)
    for c in range(NCORES):
        for b in range(BPC):
            pos = pos_list[c][b]
            r0 = b * CAP_B
            lp_full[pos] = res.results[c]["lp"][r0:r0 + len(pos)].astype(
                np.float32)
            gp_full[pos] = res.results[c]["gp"][r0:r0 + len(pos)].astype(
                np.float32)
    return (lp_full, gp_full)
